# revision 6
# baseline (speedup 1.0000x reference)
"""Trainium2 Bass kernel for 12-head MHA (B=4, S=2048, D=768), 8 NeuronCores.

Sharding: core c -> (batch b = c//2, head-group g = c%2 of 6 heads).
Each core computes its batch's attention for its 6 heads plus the partial
out-projection; the host sums the two partial outputs per batch and adds b_out.

Device dataflow keeps the sequence axis on the SBUF free dimension everywhere,
so no on-chip transposes are needed:
  QK^T proj : stationary = W columns (head-pair packed), moving = x^T chunks
  V proj    : stationary = x^T chunks, moving = W_v columns (natural V layout)
  scores^T  : stationary = K^T tile, moving = Q^T cols (two heads row-tiled)
  exp       : ScalarE from PSUM in 3-ktile windows
  attn @ V  : stationary = V tile, moving = exp'd probs (two heads col-tiled)
  denoms    : stationary = ones, same moving stream (broadcast rows for free)
  out proj  : stationary = pair-stacked context^T, moving = W_out rows
"""

import sys

sys.path.insert(0, "/opt/trn_rl_repo")

from contextlib import ExitStack

import numpy as np

import concourse.bacc as bacc
import concourse.bass as bass
import concourse.tile as tile
from concourse import mybir
from concourse.bass_utils import run_bass_kernel_spmd

F32 = mybir.dt.float32
AF = mybir.ActivationFunctionType

B, S, D = 4, 2048, 768
H, DK = 12, 64
HG = 6            # heads per core (head group)
NP = 3            # head pairs per core
NC_CHUNKS = D // 128   # 6 contraction chunks over d_model
SCH = 4           # seq chunks of 512 in phase 1
QC = 4            # q chunks of 512 in phase 2
KT = S // 128     # 16 key tiles
WIN = 3           # ktile window per exp activation


def build_nc(has_qkv_bias: bool):
    nc = bacc.Bacc("TRN2")
    xT = nc.dram_tensor("xT", [D, S], F32, kind="ExternalInput")
    wqk = nc.dram_tensor("wqk", [D, NP * 2 * 128], F32, kind="ExternalInput")
    wv = nc.dram_tensor("wv", [D, HG * DK], F32, kind="ExternalInput")
    wo = nc.dram_tensor("wo", [HG * DK, D], F32, kind="ExternalInput")
    if has_qkv_bias:
        bqk = nc.dram_tensor("bqk", [128, NP * 2], F32, kind="ExternalInput")
        bv = nc.dram_tensor("bv", [128, HG * DK], F32, kind="ExternalInput")
    out = nc.dram_tensor("out", [S, D], F32, kind="ExternalOutput")

    with tile.TileContext(nc) as tc, ExitStack() as ctx:
        singles = ctx.enter_context(tc.tile_pool(name="singles", bufs=1))
        xpool = ctx.enter_context(tc.tile_pool(name="xpool", bufs=2))
        pTpool = ctx.enter_context(tc.tile_pool(name="pTpool", bufs=2))
        opool = ctx.enter_context(tc.tile_pool(name="opool", bufs=3))
        rpool = ctx.enter_context(tc.tile_pool(name="rpool", bufs=2))

        # ---- static weights in SBUF ----
        wqk_sb = singles.tile([128, NC_CHUNKS, NP * 2 * 128], F32)
        nc.sync.dma_start(out=wqk_sb, in_=wqk.rearrange("(c p) n -> p c n", p=128))
        wv_sb = singles.tile([128, NC_CHUNKS, HG * DK], F32)
        nc.sync.dma_start(out=wv_sb, in_=wv.rearrange("(c p) n -> p c n", p=128))
        wo_sb = singles.tile([128, NP, D], F32)
        nc.sync.dma_start(out=wo_sb, in_=wo.rearrange("(c p) n -> p c n", p=128))
        if has_qkv_bias:
            bqk_sb = singles.tile([128, NP * 2], F32)
            nc.sync.dma_start(out=bqk_sb, in_=bqk[:, :])
            bv_sb = singles.tile([128, HG * DK], F32)
            nc.sync.dma_start(out=bv_sb, in_=bv[:, :])
        ones_sb = singles.tile([128, DK], F32)
        nc.vector.memset(ones_sb, 1.0)

        # persistent activations
        qkT_sb = singles.tile([128, NP, 2, S], F32)   # [.., pair, q/k, seq]
        v_sb = singles.tile([128, KT, HG * DK], F32)  # natural V per key tile
        ctx_sb = singles.tile([128, NP, S], F32)      # pair-stacked context^T

        xT_r = xT.rearrange("(c p) n -> p c n", p=128)

        # ---- phase 1: projections ----
        ph1 = ExitStack()
        ps_proj = ph1.enter_context(tc.tile_pool(name="ps_proj", bufs=2, space="PSUM"))
        for s in range(SCH):
            xt = xpool.tile([128, NC_CHUNKS, 512], F32)
            nc.sync.dma_start(out=xt, in_=xT_r[:, :, s * 512:(s + 1) * 512])
            for p3 in range(NP):
                for qk in range(2):
                    col = (p3 * 2 + qk) * 128
                    ps = ps_proj.tile([128, 512], F32, tag="ps_qk")
                    for c in range(NC_CHUNKS):
                        nc.tensor.matmul(
                            ps, wqk_sb[:, c, col:col + 128], xt[:, c, :],
                            start=(c == 0), stop=(c == NC_CHUNKS - 1),
                        )
                    dst = qkT_sb[:, p3, qk, s * 512:(s + 1) * 512]
                    if has_qkv_bias:
                        bias_col = p3 * 2 + qk
                        nc.vector.tensor_tensor(
                            dst, ps,
                            bqk_sb[:, bias_col:bias_col + 1].to_broadcast((128, 512)),
                            mybir.AluOpType.add,
                        )
                    else:
                        nc.vector.tensor_copy(dst, ps)
            for t in range(4):
                kt = s * 4 + t
                psv = ps_proj.tile([128, HG * DK], F32, tag="ps_v")
                for c in range(NC_CHUNKS):
                    nc.tensor.matmul(
                        psv, xt[:, c, t * 128:(t + 1) * 128], wv_sb[:, c, :],
                        start=(c == 0), stop=(c == NC_CHUNKS - 1),
                    )
                if has_qkv_bias:
                    nc.vector.tensor_tensor(
                        v_sb[:, kt, :], psv, bv_sb, mybir.AluOpType.add,
                    )
                else:
                    nc.vector.tensor_copy(v_sb[:, kt, :], psv)

        ph1.close()

        # ---- phase 2: attention ----
        ph2 = ExitStack()
        ps_sc = ph2.enter_context(tc.tile_pool(name="ps_sc", bufs=1, space="PSUM"))
        ps_ctx = ph2.enter_context(tc.tile_pool(name="ps_ctx", bufs=1, space="PSUM"))
        ps_den = ph2.enter_context(tc.tile_pool(name="ps_den", bufs=1, space="PSUM"))
        windows = []
        k0 = 0
        while k0 < KT:
            wl = min(WIN, KT - k0)
            windows.append((k0, wl))
            k0 += wl

        for p3 in range(NP):
            for qc in range(QC):
                qs = qc * 512
                ctx_ps = ps_ctx.tile([128, 512], F32)
                den_ps = ps_den.tile([128, 512], F32)
                for (k0, wl) in windows:
                    sc = ps_sc.tile([128, 2, WIN, 512], F32)
                    pT = pTpool.tile([128, 2, WIN, 512], F32)
                    for j in range(wl):
                        ktile = k0 + j
                        for h2 in range(2):
                            base = h2 * 64
                            nc.tensor.matmul(
                                sc[:, h2, j, :],
                                qkT_sb[base:base + 64, p3, 1,
                                       ktile * 128:(ktile + 1) * 128],
                                qkT_sb[base:base + 64, p3, 0, qs:qs + 512],
                                start=True, stop=True,
                                tile_position=(base, 0),
                            )
                    for h2 in range(2):
                        nc.scalar.activation(
                            pT[:, h2, 0:wl, :], sc[:, h2, 0:wl, :],
                            AF.Exp, scale=0.125,
                        )
                    for j in range(wl):
                        ktile = k0 + j
                        for h2 in range(2):
                            head = p3 * 2 + h2
                            nc.tensor.matmul(
                                ctx_ps[h2 * 64:(h2 + 1) * 64, :],
                                v_sb[:, ktile, head * DK:(head + 1) * DK],
                                pT[:, h2, j, :],
                                start=(ktile == 0), stop=(ktile == KT - 1),
                                tile_position=(0, h2 * 64),
                                skip_group_check=True,
                            )
                            nc.tensor.matmul(
                                den_ps[h2 * 64:(h2 + 1) * 64, :],
                                ones_sb,
                                pT[:, h2, j, :],
                                start=(ktile == 0), stop=(ktile == KT - 1),
                                tile_position=(0, h2 * 64),
                                skip_group_check=True,
                            )
                rd = rpool.tile([128, 512], F32)
                nc.vector.reciprocal(rd, den_ps)
                nc.vector.tensor_tensor(
                    ctx_sb[:, p3, qs:qs + 512], ctx_ps, rd, mybir.AluOpType.mult,
                )

        ph2.close()

        # ---- phase 3: output projection (partial; host adds pair + b_out) ----
        ph3 = ExitStack()
        ps_out = ph3.enter_context(tc.tile_pool(name="ps_out", bufs=2, space="PSUM"))
        for qt in range(S // 128):
            po = ps_out.tile([128, 2, 512], F32)
            for p3 in range(NP):
                lhsT = ctx_sb[:, p3, qt * 128:(qt + 1) * 128]
                nc.tensor.matmul(
                    po[:, 0, :], lhsT, wo_sb[:, p3, 0:512],
                    start=(p3 == 0), stop=(p3 == NP - 1),
                )
                nc.tensor.matmul(
                    po[:, 1, 0:256], lhsT, wo_sb[:, p3, 512:768],
                    start=(p3 == 0), stop=(p3 == NP - 1),
                )
            ot = opool.tile([128, D], F32)
            nc.vector.tensor_copy(ot[:, 0:512], po[:, 0, :])
            nc.vector.tensor_copy(ot[:, 512:768], po[:, 1, 0:256])
            nc.sync.dma_start(out=out[qt * 128:(qt + 1) * 128, :], in_=ot)
        ph3.close()

    nc.compile()
    return nc


_cache = {}


def _get_nc(has_qkv_bias: bool):
    if has_qkv_bias not in _cache:
        _cache[has_qkv_bias] = build_nc(has_qkv_bias)
    return _cache[has_qkv_bias]


def _prep_core_inputs(x, W_qkv, b_qkv, W_out, g):
    """Host-side shard prep for head-group g (heads g*HG .. g*HG+HG-1)."""
    heads = [g * HG + j for j in range(HG)]
    # W_qkv columns per head h: [h*192, h*192+64) = Q, +64..128 = K, +128..192 = V
    wqk_cols = []
    for p3 in range(NP):
        hA, hB = heads[2 * p3], heads[2 * p3 + 1]
        for qk in range(2):
            off = qk * DK
            wqk_cols.append(W_qkv[:, hA * 192 + off: hA * 192 + off + DK])
            wqk_cols.append(W_qkv[:, hB * 192 + off: hB * 192 + off + DK])
    wqk = np.ascontiguousarray(np.concatenate(wqk_cols, axis=1), dtype=np.float32)
    wv = np.ascontiguousarray(
        np.concatenate(
            [W_qkv[:, h * 192 + 128: h * 192 + 192] for h in heads], axis=1
        ),
        dtype=np.float32,
    )
    wo = np.ascontiguousarray(
        np.concatenate([W_out[h * DK:(h + 1) * DK, :] for h in heads], axis=0),
        dtype=np.float32,
    )
    ins = {"wqk": wqk, "wv": wv, "wo": wo}
    if b_qkv is not None:
        bqk = np.zeros((128, NP * 2), dtype=np.float32)
        for p3 in range(NP):
            hA, hB = heads[2 * p3], heads[2 * p3 + 1]
            for qk in range(2):
                off = qk * DK
                bqk[0:64, p3 * 2 + qk] = b_qkv[hA * 192 + off: hA * 192 + off + DK]
                bqk[64:128, p3 * 2 + qk] = b_qkv[hB * 192 + off: hB * 192 + off + DK]
        bv_flat = np.concatenate(
            [b_qkv[h * 192 + 128: h * 192 + 192] for h in heads]
        ).astype(np.float32)
        ins["bqk"] = bqk
        ins["bv"] = np.ascontiguousarray(np.tile(bv_flat[None, :], (128, 1)))
    return ins


def kernel(x, W_qkv, b_qkv, W_out, b_out):
    x = np.asarray(x, dtype=np.float32)
    W_qkv = np.asarray(W_qkv, dtype=np.float32)
    b_qkv = np.asarray(b_qkv, dtype=np.float32)
    W_out = np.asarray(W_out, dtype=np.float32)
    b_out = np.asarray(b_out, dtype=np.float32)

    has_bias = bool(np.any(b_qkv))
    nc = _get_nc(has_bias)

    group_ins = [
        _prep_core_inputs(x, W_qkv, b_qkv if has_bias else None, W_out, g)
        for g in range(2)
    ]
    in_maps = []
    for c in range(8):
        b, g = c // 2, c % 2
        m = dict(group_ins[g])
        m["xT"] = np.ascontiguousarray(x[b].T)
        in_maps.append(m)

    res = run_bass_kernel_spmd(nc, in_maps, list(range(8)))
    out = np.empty((B, S, D), dtype=np.float32)
    for b in range(B):
        out[b] = res.results[2 * b]["out"] + res.results[2 * b + 1]["out"] + b_out
    return out


# revision 16
# speedup vs baseline: 1.9488x; 1.9488x over previous
"""Trainium2 Bass kernel for 12-head MHA (B=4, S=2048, D=768), 8 NeuronCores.

Sharding: core c -> (batch b = c//2, head-group g = c%2 of 6 heads).
Each core computes its batch's attention for its 6 heads plus the partial
out-projection; the host sums the two partial outputs per batch and adds b_out.

Device dataflow keeps the sequence axis on the SBUF free dimension everywhere,
so no on-chip transposes are needed:
  QK^T proj : stationary = W columns (head-pair packed), moving = x^T chunks
  V proj    : stationary = x^T chunks, moving = W_v columns (natural V layout)
  scores^T  : stationary = K^T tile, moving = Q^T cols (two heads row-tiled)
  exp       : ScalarE from PSUM in 3-ktile windows
  attn @ V  : stationary = V tile, moving = exp'd probs (two heads col-tiled)
  denoms    : stationary = ones, same moving stream (broadcast rows for free)
  out proj  : stationary = pair-stacked context^T, moving = W_out rows
"""

import sys

sys.path.insert(0, "/opt/trn_rl_repo")

from contextlib import ExitStack

import numpy as np

import concourse.bacc as bacc
import concourse.bass as bass
import concourse.tile as tile
from concourse import mybir
from concourse.bass_utils import run_bass_kernel_spmd

F32 = mybir.dt.float32
AF = mybir.ActivationFunctionType

MM_DT = mybir.dt.float32r


def _mm(nc, out, lhsT, rhs, **kw):
    nc.tensor.matmul(out, lhsT, rhs, **kw)


B, S, D = 4, 2048, 768
H, DK = 12, 64
HG = 6            # heads per core (head group)
NP = 3            # head pairs per core
NC_CHUNKS = D // 128   # 6 contraction chunks over d_model
SCH = 4           # seq chunks of 512 in phase 1
QC = 4            # q chunks of 512 in phase 2
KT = S // 128     # 16 key tiles
WIN = 3           # ktile window per exp activation


def build_nc(has_qkv_bias: bool):
    nc = bacc.Bacc("TRN2")
    xT = nc.dram_tensor("xT", [D, S], MM_DT, kind="ExternalInput")
    wqk = nc.dram_tensor("wqk", [D, NP * 2 * 128], MM_DT, kind="ExternalInput")
    wv = nc.dram_tensor("wv", [D, HG * DK], MM_DT, kind="ExternalInput")
    wo = nc.dram_tensor("wo", [HG * DK, D], MM_DT, kind="ExternalInput")
    if has_qkv_bias:
        bqk = nc.dram_tensor("bqk", [128, NP * 2], F32, kind="ExternalInput")
        bv = nc.dram_tensor("bv", [128, HG * DK], F32, kind="ExternalInput")
    out = nc.dram_tensor("out", [S, D], F32, kind="ExternalOutput")
    rden_dram = nc.dram_tensor("rden_scratch", [2 * NP, S], F32)

    with tile.TileContext(nc) as tc, ExitStack() as ctx:
        singles = ctx.enter_context(tc.tile_pool(name="singles", bufs=1))
        xpool = ctx.enter_context(tc.tile_pool(name="xpool", bufs=2))
        pTpool = ctx.enter_context(tc.tile_pool(name="pTpool", bufs=2))
        opool = ctx.enter_context(tc.tile_pool(name="opool", bufs=2))
        rpool = ctx.enter_context(tc.tile_pool(name="rpool", bufs=2))

        # ---- static weights in SBUF ----
        wqk_sb = singles.tile([128, NC_CHUNKS, NP * 2 * 128], MM_DT)
        nc.sync.dma_start(out=wqk_sb, in_=wqk.rearrange("(c p) n -> p c n", p=128))
        wv_sb = singles.tile([128, NC_CHUNKS, HG * DK], MM_DT)
        nc.sync.dma_start(out=wv_sb, in_=wv.rearrange("(c p) n -> p c n", p=128))
        wo_sb = singles.tile([128, NP, D], MM_DT)
        nc.sync.dma_start(out=wo_sb, in_=wo.rearrange("(c p) n -> p c n", p=128))
        if has_qkv_bias:
            bqk_sb = singles.tile([128, NP * 2], F32)
            nc.sync.dma_start(out=bqk_sb, in_=bqk[:, :])
            bv_sb = singles.tile([128, HG * DK], F32)
            nc.sync.dma_start(out=bv_sb, in_=bv[:, :])

        # persistent activations
        qkT_sb = singles.tile([128, NP, 2, S], MM_DT)   # [.., pair, q/k, seq]
        vaug_sb = singles.tile([128, KT, HG, DK + 1], MM_DT)  # [V_h | ones]
        ctx_sb = singles.tile([128, NP, S], MM_DT)      # context^T (raw, then normalized in place)
        # fill the ones column of vaug
        ones_f = singles.tile([128, 1], F32)
        nc.vector.memset(ones_f, 1.0)
        nc.vector.tensor_copy(
            vaug_sb[:, :, :, DK:DK + 1],
            ones_f[:, None, None, :].to_broadcast((128, KT, HG, 1)),
        )

        xT_r = xT.rearrange("(c p) n -> p c n", p=128)

        # ---- phase 1: projections ----
        ph1 = ExitStack()
        ps_proj = ph1.enter_context(tc.tile_pool(name="ps_proj", bufs=2, space="PSUM"))
        for s in range(SCH):
            xt = xpool.tile([128, NC_CHUNKS, 512], MM_DT)
            nc.sync.dma_start(out=xt, in_=xT_r[:, :, s * 512:(s + 1) * 512])
            for p3 in range(NP):
                for qk in range(2):
                    col = (p3 * 2 + qk) * 128
                    ps = ps_proj.tile([128, 512], F32, tag="ps_qk")
                    for c in range(NC_CHUNKS):
                        _mm(nc, 
                            ps, wqk_sb[:, c, col:col + 128], xt[:, c, :],
                            start=(c == 0), stop=(c == NC_CHUNKS - 1),
                        )
                    dst = qkT_sb[:, p3, qk, s * 512:(s + 1) * 512]
                    if has_qkv_bias:
                        bias_col = p3 * 2 + qk
                        nc.vector.tensor_tensor(
                            dst, ps,
                            bqk_sb[:, bias_col:bias_col + 1].to_broadcast((128, 512)),
                            mybir.AluOpType.add,
                        )
                    else:
                        nc.vector.tensor_copy(dst, ps)
            for t in range(4):
                kt = s * 4 + t
                psv = ps_proj.tile([128, HG * DK], F32, tag="ps_v")
                for c in range(NC_CHUNKS):
                    _mm(nc, 
                        psv, xt[:, c, t * 128:(t + 1) * 128], wv_sb[:, c, :],
                        start=(c == 0), stop=(c == NC_CHUNKS - 1),
                    )
                vdst = vaug_sb[:, kt, :, 0:DK]
                if has_qkv_bias:
                    nc.vector.tensor_tensor(
                        vdst, psv.rearrange("p (h d) -> p h d", h=HG),
                        bv_sb.rearrange("p (h d) -> p h d", h=HG),
                        mybir.AluOpType.add,
                    )
                else:
                    nc.vector.tensor_copy(
                        vdst, psv.rearrange("p (h d) -> p h d", h=HG))

        ph1.close()

        # ---- phase 2: attention ----
        # Per head: scores^T row-tiled; exp; fused attnV+denominator matmul
        # with stationary [V_h | ones] (M=65, float32r-legal base-0 output).
        ph2 = ExitStack()
        ps_sc = ph2.enter_context(tc.tile_pool(name="ps_sc", bufs=1, space="PSUM"))
        ps_ctxA = ph2.enter_context(tc.tile_pool(name="ps_ctxA", bufs=1, space="PSUM"))
        ps_ctxB = ph2.enter_context(tc.tile_pool(name="ps_ctxB", bufs=1, space="PSUM"))
        windows = []
        k0 = 0
        while k0 < KT:
            wl = min(WIN, KT - k0)
            windows.append((k0, wl))
            k0 += wl

        for p3 in range(NP):
            for qc in range(QC):
                qs = qc * 512
                ctx_ps_a = ps_ctxA.tile([128, 512], F32)
                ctx_ps_b = ps_ctxB.tile([128, 512], F32)
                ctx_ps = [ctx_ps_a, ctx_ps_b]
                for (k0, wl) in windows:
                    sc = ps_sc.tile([128, 2, WIN, 512], F32)
                    pT = pTpool.tile([128, 2, WIN, 512], MM_DT)
                    for j in range(wl):
                        ktile = k0 + j
                        for h2 in range(2):
                            base = h2 * 64
                            _mm(nc,
                                sc[:, h2, j, :],
                                qkT_sb[base:base + 64, p3, 1,
                                       ktile * 128:(ktile + 1) * 128],
                                qkT_sb[base:base + 64, p3, 0, qs:qs + 512],
                                start=True, stop=True,
                                tile_position=(base, 0),
                            )
                    for h2 in range(2):
                        nc.scalar.activation(
                            pT[:, h2, 0:wl, :], sc[:, h2, 0:wl, :],
                            AF.Exp, scale=0.125,
                        )
                    for j in range(wl):
                        ktile = k0 + j
                        for h2 in range(2):
                            head = p3 * 2 + h2
                            _mm(nc,
                                ctx_ps[h2][0:65, :],
                                vaug_sb[:, ktile, head, :],
                                pT[:, h2, j, :],
                                start=(ktile == 0), stop=(ktile == KT - 1),
                                skip_group_check=True,
                            )
                # raw context (pair-stacked) + denominator rows
                nc.vector.tensor_copy(ctx_sb[0:64, p3, qs:qs + 512],
                                      ctx_ps[0][0:64, :])
                nc.vector.tensor_copy(ctx_sb[64:128, p3, qs:qs + 512],
                                      ctx_ps[1][0:64, :])
                dtmp_a = rpool.tile([1, 512], F32, tag="dtmp_a")
                dtmp_b = rpool.tile([1, 512], F32, tag="dtmp_b")
                nc.vector.tensor_copy(dtmp_a, ctx_ps[0][64:65, :])
                nc.vector.tensor_copy(dtmp_b, ctx_ps[1][64:65, :])
                nc.sync.dma_start(out=rden_dram[2 * p3:2 * p3 + 1, qs:qs + 512],
                                  in_=dtmp_a)
                nc.sync.dma_start(out=rden_dram[2 * p3 + 1:2 * p3 + 2, qs:qs + 512],
                                  in_=dtmp_b)

        # ---- phase 2.5: softmax normalization ----
        for p3 in range(NP):
            for qc in range(QC):
                qs = qc * 512
                rbc = rpool.tile([128, 512], F32, tag="rbc")
                for h2 in range(2):
                    row = rden_dram[2 * p3 + h2:2 * p3 + h2 + 1, qs:qs + 512]
                    bcast = bass.AP(tensor=row.tensor, offset=row.offset,
                                    ap=[[0, 64]] + row.ap[1:])
                    nc.sync.dma_start(out=rbc[h2 * 64:(h2 + 1) * 64, :], in_=bcast)
                nc.vector.reciprocal_approx_fast(rbc, rbc)
                nc.vector.tensor_tensor(
                    ctx_sb[:, p3, qs:qs + 512],
                    ctx_sb[:, p3, qs:qs + 512],
                    rbc, mybir.AluOpType.mult,
                )

        ph2.close()

        # ---- phase 3: output projection (partial; host adds pair + b_out) ----
        ph3 = ExitStack()
        ps_out = ph3.enter_context(tc.tile_pool(name="ps_out", bufs=2, space="PSUM"))
        for qt in range(S // 128):
            po = ps_out.tile([128, 2, 512], F32)
            for p3 in range(NP):
                lhsT = ctx_sb[:, p3, qt * 128:(qt + 1) * 128]
                _mm(nc, 
                    po[:, 0, :], lhsT, wo_sb[:, p3, 0:512],
                    start=(p3 == 0), stop=(p3 == NP - 1),
                )
                _mm(nc, 
                    po[:, 1, 0:256], lhsT, wo_sb[:, p3, 512:768],
                    start=(p3 == 0), stop=(p3 == NP - 1),
                )
            ot = opool.tile([128, D], F32)
            nc.vector.tensor_copy(ot[:, 0:512], po[:, 0, :])
            nc.vector.tensor_copy(ot[:, 512:768], po[:, 1, 0:256])
            nc.sync.dma_start(out=out[qt * 128:(qt + 1) * 128, :], in_=ot)
        ph3.close()

    nc.compile()
    return nc


_cache = {}


def _get_nc(has_qkv_bias: bool):
    if has_qkv_bias not in _cache:
        _cache[has_qkv_bias] = build_nc(has_qkv_bias)
    return _cache[has_qkv_bias]


def _prep_core_inputs(x, W_qkv, b_qkv, W_out, g):
    """Host-side shard prep for head-group g (heads g*HG .. g*HG+HG-1)."""
    heads = [g * HG + j for j in range(HG)]
    # W_qkv columns per head h: [h*192, h*192+64) = Q, +64..128 = K, +128..192 = V
    wqk_cols = []
    for p3 in range(NP):
        hA, hB = heads[2 * p3], heads[2 * p3 + 1]
        for qk in range(2):
            off = qk * DK
            wqk_cols.append(W_qkv[:, hA * 192 + off: hA * 192 + off + DK])
            wqk_cols.append(W_qkv[:, hB * 192 + off: hB * 192 + off + DK])
    wqk = np.ascontiguousarray(np.concatenate(wqk_cols, axis=1), dtype=np.float32)
    wv = np.ascontiguousarray(
        np.concatenate(
            [W_qkv[:, h * 192 + 128: h * 192 + 192] for h in heads], axis=1
        ),
        dtype=np.float32,
    )
    wo = np.ascontiguousarray(
        np.concatenate([W_out[h * DK:(h + 1) * DK, :] for h in heads], axis=0),
        dtype=np.float32,
    )
    ins = {"wqk": wqk, "wv": wv, "wo": wo}
    if b_qkv is not None:
        bqk = np.zeros((128, NP * 2), dtype=np.float32)
        for p3 in range(NP):
            hA, hB = heads[2 * p3], heads[2 * p3 + 1]
            for qk in range(2):
                off = qk * DK
                bqk[0:64, p3 * 2 + qk] = b_qkv[hA * 192 + off: hA * 192 + off + DK]
                bqk[64:128, p3 * 2 + qk] = b_qkv[hB * 192 + off: hB * 192 + off + DK]
        bv_flat = np.concatenate(
            [b_qkv[h * 192 + 128: h * 192 + 192] for h in heads]
        ).astype(np.float32)
        ins["bqk"] = bqk
        ins["bv"] = np.ascontiguousarray(np.tile(bv_flat[None, :], (128, 1)))
    return ins


def kernel(x, W_qkv, b_qkv, W_out, b_out):
    x = np.asarray(x, dtype=np.float32)
    W_qkv = np.asarray(W_qkv, dtype=np.float32)
    b_qkv = np.asarray(b_qkv, dtype=np.float32)
    W_out = np.asarray(W_out, dtype=np.float32)
    b_out = np.asarray(b_out, dtype=np.float32)

    has_bias = bool(np.any(b_qkv))
    nc = _get_nc(has_bias)

    group_ins = [
        _prep_core_inputs(x, W_qkv, b_qkv if has_bias else None, W_out, g)
        for g in range(2)
    ]
    in_maps = []
    for c in range(8):
        b, g = c // 2, c % 2
        m = dict(group_ins[g])
        m["xT"] = np.ascontiguousarray(x[b].T)
        in_maps.append(m)

    res = run_bass_kernel_spmd(nc, in_maps, list(range(8)))
    out = np.empty((B, S, D), dtype=np.float32)
    for b in range(B):
        out[b] = res.results[2 * b]["out"] + res.results[2 * b + 1]["out"] + b_out
    return out


# revision 17
# speedup vs baseline: 2.5787x; 1.3232x over previous
"""Trainium2 Bass kernel for 12-head MHA (B=4, S=2048, D=768), 8 NeuronCores.

Sharding: core c -> (batch b = c//2, head-group g = c%2 of 6 heads).
Each core computes its batch's attention for its 6 heads plus the partial
out-projection; the host sums the two partial outputs per batch and adds b_out.

Device dataflow keeps the sequence axis on the SBUF free dimension everywhere,
so no on-chip transposes are needed:
  QK^T proj : stationary = W columns (head-pair packed), moving = x^T chunks
  V proj    : stationary = x^T chunks, moving = W_v columns (natural V layout)
  scores^T  : stationary = K^T tile, moving = Q^T cols (two heads row-tiled)
  exp       : ScalarE from PSUM in 3-ktile windows
  attn @ V  : stationary = V tile, moving = exp'd probs (two heads col-tiled)
  denoms    : stationary = ones, same moving stream (broadcast rows for free)
  out proj  : stationary = pair-stacked context^T, moving = W_out rows
"""

import sys

sys.path.insert(0, "/opt/trn_rl_repo")

from contextlib import ExitStack

import numpy as np

import concourse.bacc as bacc
import concourse.bass as bass
import concourse.tile as tile
from concourse import mybir
from concourse.bass_utils import run_bass_kernel_spmd

F32 = mybir.dt.float32
AF = mybir.ActivationFunctionType

MM_DT = mybir.dt.float32r


def _mm(nc, out, lhsT, rhs, **kw):
    nc.tensor.matmul(out, lhsT, rhs, **kw)


B, S, D = 4, 2048, 768
H, DK = 12, 64
HG = 6            # heads per core (head group)
NP = 3            # head pairs per core
NC_CHUNKS = D // 128   # 6 contraction chunks over d_model
SCH = 4           # seq chunks of 512 in phase 1
QC = 4            # q chunks of 512 in phase 2
KT = S // 128     # 16 key tiles
WIN = 3           # ktile window per exp activation


def build_nc(has_qkv_bias: bool):
    nc = bacc.Bacc("TRN2")
    xT = nc.dram_tensor("xT", [D, S], MM_DT, kind="ExternalInput")
    wqk = nc.dram_tensor("wqk", [D, NP * 2 * 128], MM_DT, kind="ExternalInput")
    wv = nc.dram_tensor("wv", [D, HG * DK], MM_DT, kind="ExternalInput")
    wo = nc.dram_tensor("wo", [HG * DK, D], MM_DT, kind="ExternalInput")
    if has_qkv_bias:
        bqk = nc.dram_tensor("bqk", [128, NP * 2], F32, kind="ExternalInput")
        bv = nc.dram_tensor("bv", [128, HG * DK], F32, kind="ExternalInput")
    out = nc.dram_tensor("out", [S, D], F32, kind="ExternalOutput")
    rden_dram = nc.dram_tensor("rden_scratch", [2 * NP, S], F32)

    with tile.TileContext(nc) as tc, ExitStack() as ctx:
        singles = ctx.enter_context(tc.tile_pool(name="singles", bufs=1))
        xpool = ctx.enter_context(tc.tile_pool(name="xpool", bufs=2))
        pTpool = ctx.enter_context(tc.tile_pool(name="pTpool", bufs=2))
        opool = ctx.enter_context(tc.tile_pool(name="opool", bufs=2))
        rpool = ctx.enter_context(tc.tile_pool(name="rpool", bufs=2))

        # ---- static weights in SBUF ----
        wqk_sb = singles.tile([128, NC_CHUNKS, NP * 2 * 128], MM_DT)
        nc.sync.dma_start(out=wqk_sb, in_=wqk.rearrange("(c p) n -> p c n", p=128))
        wv_sb = singles.tile([128, NC_CHUNKS, HG * DK], MM_DT)
        nc.sync.dma_start(out=wv_sb, in_=wv.rearrange("(c p) n -> p c n", p=128))
        wo_sb = singles.tile([128, NP, D], MM_DT)
        nc.sync.dma_start(out=wo_sb, in_=wo.rearrange("(c p) n -> p c n", p=128))
        if has_qkv_bias:
            bqk_sb = singles.tile([128, NP * 2], F32)
            nc.sync.dma_start(out=bqk_sb, in_=bqk[:, :])
            bv_sb = singles.tile([128, HG * DK], F32)
            nc.sync.dma_start(out=bv_sb, in_=bv[:, :])

        # persistent activations
        qkT_sb = singles.tile([128, NP, 2, S], MM_DT)   # [.., pair, q/k, seq]
        vaug_sb = singles.tile([128, KT, HG, DK + 1], MM_DT)  # [V_h | ones]
        ctx_sb = singles.tile([128, NP, S], MM_DT)      # context^T (raw, then normalized in place)
        # fill the ones column of vaug
        ones_f = singles.tile([128, 1], F32)
        nc.vector.memset(ones_f, 1.0)
        nc.vector.tensor_copy(
            vaug_sb[:, :, :, DK:DK + 1],
            ones_f[:, None, None, :].to_broadcast((128, KT, HG, 1)),
        )

        xT_r = xT.rearrange("(c p) n -> p c n", p=128)

        # ---- phase 1: projections ----
        ph1 = ExitStack()
        ps_proj = ph1.enter_context(tc.tile_pool(name="ps_proj", bufs=2, space="PSUM"))
        for s in range(SCH):
            xt = xpool.tile([128, NC_CHUNKS, 512], MM_DT)
            nc.sync.dma_start(out=xt, in_=xT_r[:, :, s * 512:(s + 1) * 512])
            for p3 in range(NP):
                for qk in range(2):
                    col = (p3 * 2 + qk) * 128
                    ps = ps_proj.tile([128, 512], F32, tag="ps_qk")
                    for c in range(NC_CHUNKS):
                        _mm(nc, 
                            ps, wqk_sb[:, c, col:col + 128], xt[:, c, :],
                            start=(c == 0), stop=(c == NC_CHUNKS - 1),
                        )
                    dst = qkT_sb[:, p3, qk, s * 512:(s + 1) * 512]
                    if has_qkv_bias:
                        bias_col = p3 * 2 + qk
                        nc.vector.tensor_tensor(
                            dst, ps,
                            bqk_sb[:, bias_col:bias_col + 1].to_broadcast((128, 512)),
                            mybir.AluOpType.add,
                        )
                    else:
                        nc.vector.tensor_copy(dst, ps)
            for t in range(4):
                kt = s * 4 + t
                psv = ps_proj.tile([128, HG * DK], F32, tag="ps_v")
                for c in range(NC_CHUNKS):
                    _mm(nc, 
                        psv, xt[:, c, t * 128:(t + 1) * 128], wv_sb[:, c, :],
                        start=(c == 0), stop=(c == NC_CHUNKS - 1),
                    )
                vdst = vaug_sb[:, kt, :, 0:DK]
                if has_qkv_bias:
                    nc.vector.tensor_tensor(
                        vdst, psv.rearrange("p (h d) -> p h d", h=HG),
                        bv_sb.rearrange("p (h d) -> p h d", h=HG),
                        mybir.AluOpType.add,
                    )
                else:
                    nc.vector.tensor_copy(
                        vdst, psv.rearrange("p (h d) -> p h d", h=HG))

        ph1.close()

        # ---- phase 2: attention + out-projection, pipelined per q-chunk ----
        # WIN=1 double-buffered score tensors keep PE gaps short (HAM stays
        # warm); out-projection for each q-chunk is folded in so PE has work
        # while ScalarE runs exp.
        ph2 = ExitStack()
        ps_sc = ph2.enter_context(tc.tile_pool(name="ps_sc", bufs=2, space="PSUM"))
        ps_ctxA = ph2.enter_context(tc.tile_pool(name="ps_ctxA", bufs=1, space="PSUM"))
        ps_ctxB = ph2.enter_context(tc.tile_pool(name="ps_ctxB", bufs=1, space="PSUM"))
        ps_out = ph2.enter_context(tc.tile_pool(name="ps_out", bufs=1, space="PSUM"))

        for qc in range(QC):
            qs = qc * 512
            for p3 in range(NP):
                ctx_ps_a = ps_ctxA.tile([128, 512], F32)
                ctx_ps_b = ps_ctxB.tile([128, 512], F32)
                ctx_ps = [ctx_ps_a, ctx_ps_b]
                for ktile in range(KT):
                    sc = ps_sc.tile([128, 2, 512], F32)
                    pT = pTpool.tile([128, 2, 512], MM_DT)
                    for h2 in range(2):
                        base = h2 * 64
                        _mm(nc,
                            sc[:, h2, :],
                            qkT_sb[base:base + 64, p3, 1,
                                   ktile * 128:(ktile + 1) * 128],
                            qkT_sb[base:base + 64, p3, 0, qs:qs + 512],
                            start=True, stop=True,
                            tile_position=(base, 0),
                        )
                    nc.scalar.activation(pT, sc, AF.Exp, scale=0.125)
                    for h2 in range(2):
                        head = p3 * 2 + h2
                        _mm(nc,
                            ctx_ps[h2][0:65, :],
                            vaug_sb[:, ktile, head, :],
                            pT[:, h2, :],
                            start=(ktile == 0), stop=(ktile == KT - 1),
                            skip_group_check=True,
                        )
                # raw context (pair-stacked) + denominator rows
                nc.vector.tensor_copy(ctx_sb[0:64, p3, qs:qs + 512],
                                      ctx_ps[0][0:64, :])
                nc.vector.tensor_copy(ctx_sb[64:128, p3, qs:qs + 512],
                                      ctx_ps[1][0:64, :])
                dtmp_a = rpool.tile([1, 512], F32, tag="dtmp_a")
                dtmp_b = rpool.tile([1, 512], F32, tag="dtmp_b")
                nc.vector.tensor_copy(dtmp_a, ctx_ps[0][64:65, :])
                nc.vector.tensor_copy(dtmp_b, ctx_ps[1][64:65, :])
                nc.sync.dma_start(out=rden_dram[2 * p3:2 * p3 + 1, qs:qs + 512],
                                  in_=dtmp_a)
                nc.sync.dma_start(out=rden_dram[2 * p3 + 1:2 * p3 + 2, qs:qs + 512],
                                  in_=dtmp_b)

            # normalization for this q-chunk
            for p3 in range(NP):
                rbc = rpool.tile([128, 512], F32, tag="rbc")
                for h2 in range(2):
                    row = rden_dram[2 * p3 + h2:2 * p3 + h2 + 1, qs:qs + 512]
                    bcast = bass.AP(tensor=row.tensor, offset=row.offset,
                                    ap=[[0, 64]] + row.ap[1:])
                    nc.sync.dma_start(out=rbc[h2 * 64:(h2 + 1) * 64, :], in_=bcast)
                nc.vector.reciprocal_approx_fast(rbc, rbc)
                nc.vector.tensor_tensor(
                    ctx_sb[:, p3, qs:qs + 512],
                    ctx_sb[:, p3, qs:qs + 512],
                    rbc, mybir.AluOpType.mult,
                )

            # out-projection for this q-chunk (partial over this head group)
            for qt in range(4):
                qtg = qc * 4 + qt
                po = ps_out.tile([128, 2, 512], F32)
                for p3 in range(NP):
                    lhsT = ctx_sb[:, p3, qtg * 128:(qtg + 1) * 128]
                    _mm(nc,
                        po[:, 0, :], lhsT, wo_sb[:, p3, 0:512],
                        start=(p3 == 0), stop=(p3 == NP - 1),
                    )
                    _mm(nc,
                        po[:, 1, 0:256], lhsT, wo_sb[:, p3, 512:768],
                        start=(p3 == 0), stop=(p3 == NP - 1),
                    )
                ot = opool.tile([128, D], F32)
                nc.vector.tensor_copy(ot[:, 0:512], po[:, 0, :])
                nc.vector.tensor_copy(ot[:, 512:768], po[:, 1, 0:256])
                nc.sync.dma_start(out=out[qtg * 128:(qtg + 1) * 128, :], in_=ot)
        ph2.close()

    nc.compile()
    return nc


_cache = {}


def _get_nc(has_qkv_bias: bool):
    if has_qkv_bias not in _cache:
        _cache[has_qkv_bias] = build_nc(has_qkv_bias)
    return _cache[has_qkv_bias]


def _prep_core_inputs(x, W_qkv, b_qkv, W_out, g):
    """Host-side shard prep for head-group g (heads g*HG .. g*HG+HG-1)."""
    heads = [g * HG + j for j in range(HG)]
    # W_qkv columns per head h: [h*192, h*192+64) = Q, +64..128 = K, +128..192 = V
    wqk_cols = []
    for p3 in range(NP):
        hA, hB = heads[2 * p3], heads[2 * p3 + 1]
        for qk in range(2):
            off = qk * DK
            wqk_cols.append(W_qkv[:, hA * 192 + off: hA * 192 + off + DK])
            wqk_cols.append(W_qkv[:, hB * 192 + off: hB * 192 + off + DK])
    wqk = np.ascontiguousarray(np.concatenate(wqk_cols, axis=1), dtype=np.float32)
    wv = np.ascontiguousarray(
        np.concatenate(
            [W_qkv[:, h * 192 + 128: h * 192 + 192] for h in heads], axis=1
        ),
        dtype=np.float32,
    )
    wo = np.ascontiguousarray(
        np.concatenate([W_out[h * DK:(h + 1) * DK, :] for h in heads], axis=0),
        dtype=np.float32,
    )
    ins = {"wqk": wqk, "wv": wv, "wo": wo}
    if b_qkv is not None:
        bqk = np.zeros((128, NP * 2), dtype=np.float32)
        for p3 in range(NP):
            hA, hB = heads[2 * p3], heads[2 * p3 + 1]
            for qk in range(2):
                off = qk * DK
                bqk[0:64, p3 * 2 + qk] = b_qkv[hA * 192 + off: hA * 192 + off + DK]
                bqk[64:128, p3 * 2 + qk] = b_qkv[hB * 192 + off: hB * 192 + off + DK]
        bv_flat = np.concatenate(
            [b_qkv[h * 192 + 128: h * 192 + 192] for h in heads]
        ).astype(np.float32)
        ins["bqk"] = bqk
        ins["bv"] = np.ascontiguousarray(np.tile(bv_flat[None, :], (128, 1)))
    return ins


def kernel(x, W_qkv, b_qkv, W_out, b_out):
    x = np.asarray(x, dtype=np.float32)
    W_qkv = np.asarray(W_qkv, dtype=np.float32)
    b_qkv = np.asarray(b_qkv, dtype=np.float32)
    W_out = np.asarray(W_out, dtype=np.float32)
    b_out = np.asarray(b_out, dtype=np.float32)

    has_bias = bool(np.any(b_qkv))
    nc = _get_nc(has_bias)

    group_ins = [
        _prep_core_inputs(x, W_qkv, b_qkv if has_bias else None, W_out, g)
        for g in range(2)
    ]
    in_maps = []
    for c in range(8):
        b, g = c // 2, c % 2
        m = dict(group_ins[g])
        m["xT"] = np.ascontiguousarray(x[b].T)
        in_maps.append(m)

    res = run_bass_kernel_spmd(nc, in_maps, list(range(8)))
    out = np.empty((B, S, D), dtype=np.float32)
    for b in range(B):
        out[b] = res.results[2 * b]["out"] + res.results[2 * b + 1]["out"] + b_out
    return out


# revision 18
# speedup vs baseline: 2.7988x; 1.0854x over previous
"""Trainium2 Bass kernel for 12-head MHA (B=4, S=2048, D=768), 8 NeuronCores.

Sharding: core c -> (batch b = c//2, head-group g = c%2 of 6 heads).
Each core computes its batch's attention for its 6 heads plus the partial
out-projection; the host sums the two partial outputs per batch and adds b_out.

Device dataflow keeps the sequence axis on the SBUF free dimension everywhere,
so no on-chip transposes are needed:
  QK^T proj : stationary = W columns (head-pair packed), moving = x^T chunks
  V proj    : stationary = x^T chunks, moving = W_v columns (natural V layout)
  scores^T  : stationary = K^T tile, moving = Q^T cols (two heads row-tiled)
  exp       : ScalarE from PSUM in 3-ktile windows
  attn @ V  : stationary = V tile, moving = exp'd probs (two heads col-tiled)
  denoms    : stationary = ones, same moving stream (broadcast rows for free)
  out proj  : stationary = pair-stacked context^T, moving = W_out rows
"""

import sys

sys.path.insert(0, "/opt/trn_rl_repo")

from contextlib import ExitStack

import numpy as np

import concourse.bacc as bacc
import concourse.bass as bass
import concourse.tile as tile
from concourse import mybir
from concourse.bass_utils import run_bass_kernel_spmd

F32 = mybir.dt.float32
BF16 = mybir.dt.bfloat16
AF = mybir.ActivationFunctionType

MM_DT = mybir.dt.float32r


def _mm(nc, out, lhsT, rhs, **kw):
    nc.tensor.matmul(out, lhsT, rhs, **kw)


B, S, D = 4, 2048, 768
H, DK = 12, 64
HG = 6            # heads per core (head group)
NP = 3            # head pairs per core
NC_CHUNKS = D // 128   # 6 contraction chunks over d_model
SCH = 4           # seq chunks of 512 in phase 1
QC = 4            # q chunks of 512 in phase 2
KT = S // 128     # 16 key tiles
WIN = 3           # ktile window per exp activation


def build_nc(has_qkv_bias: bool):
    nc = bacc.Bacc("TRN2")
    xT = nc.dram_tensor("xT", [D, S], MM_DT, kind="ExternalInput")
    wqk = nc.dram_tensor("wqk", [D, NP * 2 * 128], MM_DT, kind="ExternalInput")
    wv = nc.dram_tensor("wv", [D, HG * DK], MM_DT, kind="ExternalInput")
    wo = nc.dram_tensor("wo", [HG * DK, D], MM_DT, kind="ExternalInput")
    if has_qkv_bias:
        bqk = nc.dram_tensor("bqk", [128, NP * 2], F32, kind="ExternalInput")
        bv = nc.dram_tensor("bv", [128, HG * DK], F32, kind="ExternalInput")
    out = nc.dram_tensor("out", [S, D], F32, kind="ExternalOutput")
    rden_dram = nc.dram_tensor("rden_scratch", [2 * NP, S], F32)

    with tile.TileContext(nc) as tc, ExitStack() as ctx:
        singles = ctx.enter_context(tc.tile_pool(name="singles", bufs=1))
        xpool = ctx.enter_context(tc.tile_pool(name="xpool", bufs=2))
        pTpool = ctx.enter_context(tc.tile_pool(name="pTpool", bufs=2))
        opool = ctx.enter_context(tc.tile_pool(name="opool", bufs=2))
        rpool = ctx.enter_context(tc.tile_pool(name="rpool", bufs=2))

        # ---- static weights in SBUF ----
        wqk_sb = singles.tile([128, NC_CHUNKS, NP * 2 * 128], MM_DT)
        nc.sync.dma_start(out=wqk_sb, in_=wqk.rearrange("(c p) n -> p c n", p=128))
        wv_sb = singles.tile([128, NC_CHUNKS, HG * DK], MM_DT)
        nc.sync.dma_start(out=wv_sb, in_=wv.rearrange("(c p) n -> p c n", p=128))
        wo_sb = singles.tile([128, NP, D], MM_DT)
        nc.sync.dma_start(out=wo_sb, in_=wo.rearrange("(c p) n -> p c n", p=128))
        if has_qkv_bias:
            bqk_sb = singles.tile([128, NP * 2], F32)
            nc.sync.dma_start(out=bqk_sb, in_=bqk[:, :])
            bv_sb = singles.tile([128, HG * DK], F32)
            nc.sync.dma_start(out=bv_sb, in_=bv[:, :])

        # persistent activations
        qkT_sb = singles.tile([128, NP, 2, S], MM_DT)   # [.., pair, q/k, seq]
        vaug_sb = singles.tile([128, KT, HG, DK + 1], BF16)  # [V_h | ones]
        ctx_sb = singles.tile([128, NP, S], MM_DT)      # context^T (raw, then normalized in place)
        # fill the ones column of vaug
        ones_f = singles.tile([128, 1], F32)
        nc.vector.memset(ones_f, 1.0)
        nc.vector.tensor_copy(
            vaug_sb[:, :, :, DK:DK + 1],
            ones_f[:, None, None, :].to_broadcast((128, KT, HG, 1)),
        )

        xT_r = xT.rearrange("(c p) n -> p c n", p=128)

        # ---- phase 1: projections ----
        ph1 = ExitStack()
        ps_proj = ph1.enter_context(tc.tile_pool(name="ps_proj", bufs=2, space="PSUM"))
        for s in range(SCH):
            xt = xpool.tile([128, NC_CHUNKS, 512], MM_DT)
            nc.sync.dma_start(out=xt, in_=xT_r[:, :, s * 512:(s + 1) * 512])
            for p3 in range(NP):
                for qk in range(2):
                    col = (p3 * 2 + qk) * 128
                    ps = ps_proj.tile([128, 512], F32, tag="ps_qk")
                    for c in range(NC_CHUNKS):
                        _mm(nc, 
                            ps, wqk_sb[:, c, col:col + 128], xt[:, c, :],
                            start=(c == 0), stop=(c == NC_CHUNKS - 1),
                        )
                    dst = qkT_sb[:, p3, qk, s * 512:(s + 1) * 512]
                    if has_qkv_bias:
                        bias_col = p3 * 2 + qk
                        nc.vector.tensor_tensor(
                            dst, ps,
                            bqk_sb[:, bias_col:bias_col + 1].to_broadcast((128, 512)),
                            mybir.AluOpType.add,
                        )
                    else:
                        nc.vector.tensor_copy(dst, ps)
            for t in range(4):
                kt = s * 4 + t
                psv = ps_proj.tile([128, HG * DK], F32, tag="ps_v")
                for c in range(NC_CHUNKS):
                    _mm(nc, 
                        psv, xt[:, c, t * 128:(t + 1) * 128], wv_sb[:, c, :],
                        start=(c == 0), stop=(c == NC_CHUNKS - 1),
                    )
                vdst = vaug_sb[:, kt, :, 0:DK]
                if has_qkv_bias:
                    nc.vector.tensor_tensor(
                        vdst, psv.rearrange("p (h d) -> p h d", h=HG),
                        bv_sb.rearrange("p (h d) -> p h d", h=HG),
                        mybir.AluOpType.add,
                    )
                else:
                    nc.vector.tensor_copy(
                        vdst, psv.rearrange("p (h d) -> p h d", h=HG))

        ph1.close()

        # ---- phase 2: attention + out-projection, pipelined per q-chunk ----
        # WIN=1 double-buffered score tensors keep PE gaps short (HAM stays
        # warm); out-projection for each q-chunk is folded in so PE has work
        # while ScalarE runs exp.
        ph2 = ExitStack()
        ps_sc = ph2.enter_context(tc.tile_pool(name="ps_sc", bufs=2, space="PSUM"))
        ps_ctxA = ph2.enter_context(tc.tile_pool(name="ps_ctxA", bufs=1, space="PSUM"))
        ps_ctxB = ph2.enter_context(tc.tile_pool(name="ps_ctxB", bufs=1, space="PSUM"))
        ps_out = ph2.enter_context(tc.tile_pool(name="ps_out", bufs=1, space="PSUM"))

        for qc in range(QC):
            qs = qc * 512
            for p3 in range(NP):
                ctx_ps_a = ps_ctxA.tile([128, 512], F32)
                ctx_ps_b = ps_ctxB.tile([128, 512], F32)
                ctx_ps = [ctx_ps_a, ctx_ps_b]
                for ktile in range(KT):
                    sc = ps_sc.tile([128, 2, 512], F32)
                    pT = pTpool.tile([128, 2, 512], BF16)
                    for h2 in range(2):
                        base = h2 * 64
                        _mm(nc,
                            sc[:, h2, :],
                            qkT_sb[base:base + 64, p3, 1,
                                   ktile * 128:(ktile + 1) * 128],
                            qkT_sb[base:base + 64, p3, 0, qs:qs + 512],
                            start=True, stop=True,
                            tile_position=(base, 0),
                        )
                    nc.scalar.activation(pT, sc, AF.Exp, scale=0.125)
                    for h2 in range(2):
                        head = p3 * 2 + h2
                        _mm(nc,
                            ctx_ps[h2][0:65, :],
                            vaug_sb[:, ktile, head, :],
                            pT[:, h2, :],
                            start=(ktile == 0), stop=(ktile == KT - 1),
                            skip_group_check=True,
                        )
                # raw context (pair-stacked) + denominator rows
                nc.vector.tensor_copy(ctx_sb[0:64, p3, qs:qs + 512],
                                      ctx_ps[0][0:64, :])
                nc.vector.tensor_copy(ctx_sb[64:128, p3, qs:qs + 512],
                                      ctx_ps[1][0:64, :])
                dtmp_a = rpool.tile([1, 512], F32, tag="dtmp_a")
                dtmp_b = rpool.tile([1, 512], F32, tag="dtmp_b")
                nc.vector.tensor_copy(dtmp_a, ctx_ps[0][64:65, :])
                nc.vector.tensor_copy(dtmp_b, ctx_ps[1][64:65, :])
                nc.sync.dma_start(out=rden_dram[2 * p3:2 * p3 + 1, qs:qs + 512],
                                  in_=dtmp_a)
                nc.sync.dma_start(out=rden_dram[2 * p3 + 1:2 * p3 + 2, qs:qs + 512],
                                  in_=dtmp_b)

            # normalization for this q-chunk
            for p3 in range(NP):
                rbc = rpool.tile([128, 512], F32, tag="rbc")
                for h2 in range(2):
                    row = rden_dram[2 * p3 + h2:2 * p3 + h2 + 1, qs:qs + 512]
                    bcast = bass.AP(tensor=row.tensor, offset=row.offset,
                                    ap=[[0, 64]] + row.ap[1:])
                    nc.sync.dma_start(out=rbc[h2 * 64:(h2 + 1) * 64, :], in_=bcast)
                nc.vector.reciprocal_approx_fast(rbc, rbc)
                nc.vector.tensor_tensor(
                    ctx_sb[:, p3, qs:qs + 512],
                    ctx_sb[:, p3, qs:qs + 512],
                    rbc, mybir.AluOpType.mult,
                )

            # out-projection for this q-chunk (partial over this head group)
            for qt in range(4):
                qtg = qc * 4 + qt
                po = ps_out.tile([128, 2, 512], F32)
                for p3 in range(NP):
                    lhsT = ctx_sb[:, p3, qtg * 128:(qtg + 1) * 128]
                    _mm(nc,
                        po[:, 0, :], lhsT, wo_sb[:, p3, 0:512],
                        start=(p3 == 0), stop=(p3 == NP - 1),
                    )
                    _mm(nc,
                        po[:, 1, 0:256], lhsT, wo_sb[:, p3, 512:768],
                        start=(p3 == 0), stop=(p3 == NP - 1),
                    )
                ot = opool.tile([128, D], F32)
                nc.vector.tensor_copy(ot[:, 0:512], po[:, 0, :])
                nc.vector.tensor_copy(ot[:, 512:768], po[:, 1, 0:256])
                nc.sync.dma_start(out=out[qtg * 128:(qtg + 1) * 128, :], in_=ot)
        ph2.close()

    nc.compile()
    return nc


_cache = {}


def _get_nc(has_qkv_bias: bool):
    if has_qkv_bias not in _cache:
        _cache[has_qkv_bias] = build_nc(has_qkv_bias)
    return _cache[has_qkv_bias]


def _prep_core_inputs(x, W_qkv, b_qkv, W_out, g):
    """Host-side shard prep for head-group g (heads g*HG .. g*HG+HG-1)."""
    heads = [g * HG + j for j in range(HG)]
    # W_qkv columns per head h: [h*192, h*192+64) = Q, +64..128 = K, +128..192 = V
    wqk_cols = []
    for p3 in range(NP):
        hA, hB = heads[2 * p3], heads[2 * p3 + 1]
        for qk in range(2):
            off = qk * DK
            wqk_cols.append(W_qkv[:, hA * 192 + off: hA * 192 + off + DK])
            wqk_cols.append(W_qkv[:, hB * 192 + off: hB * 192 + off + DK])
    wqk = np.ascontiguousarray(np.concatenate(wqk_cols, axis=1), dtype=np.float32)
    wv = np.ascontiguousarray(
        np.concatenate(
            [W_qkv[:, h * 192 + 128: h * 192 + 192] for h in heads], axis=1
        ),
        dtype=np.float32,
    )
    wo = np.ascontiguousarray(
        np.concatenate([W_out[h * DK:(h + 1) * DK, :] for h in heads], axis=0),
        dtype=np.float32,
    )
    ins = {"wqk": wqk, "wv": wv, "wo": wo}
    if b_qkv is not None:
        bqk = np.zeros((128, NP * 2), dtype=np.float32)
        for p3 in range(NP):
            hA, hB = heads[2 * p3], heads[2 * p3 + 1]
            for qk in range(2):
                off = qk * DK
                bqk[0:64, p3 * 2 + qk] = b_qkv[hA * 192 + off: hA * 192 + off + DK]
                bqk[64:128, p3 * 2 + qk] = b_qkv[hB * 192 + off: hB * 192 + off + DK]
        bv_flat = np.concatenate(
            [b_qkv[h * 192 + 128: h * 192 + 192] for h in heads]
        ).astype(np.float32)
        ins["bqk"] = bqk
        ins["bv"] = np.ascontiguousarray(np.tile(bv_flat[None, :], (128, 1)))
    return ins


def kernel(x, W_qkv, b_qkv, W_out, b_out):
    x = np.asarray(x, dtype=np.float32)
    W_qkv = np.asarray(W_qkv, dtype=np.float32)
    b_qkv = np.asarray(b_qkv, dtype=np.float32)
    W_out = np.asarray(W_out, dtype=np.float32)
    b_out = np.asarray(b_out, dtype=np.float32)

    has_bias = bool(np.any(b_qkv))
    nc = _get_nc(has_bias)

    group_ins = [
        _prep_core_inputs(x, W_qkv, b_qkv if has_bias else None, W_out, g)
        for g in range(2)
    ]
    in_maps = []
    for c in range(8):
        b, g = c // 2, c % 2
        m = dict(group_ins[g])
        m["xT"] = np.ascontiguousarray(x[b].T)
        in_maps.append(m)

    res = run_bass_kernel_spmd(nc, in_maps, list(range(8)))
    out = np.empty((B, S, D), dtype=np.float32)
    for b in range(B):
        out[b] = res.results[2 * b]["out"] + res.results[2 * b + 1]["out"] + b_out
    return out
